# revision 1
# baseline (speedup 1.0000x reference)
"""Trainium2 Bass kernel for nn_GATRecommender (8 NeuronCores).

Sharding strategy:
  - Encoders + fusion MLP: data-parallel over the batch (128 rows/core).
  - GAT layer 1 (8 heads): one head per core; node features replicated.
  - GAT layer 2 (1 head): contraction over the 6144 hidden dim sharded by
    head (matmul partials AllReduce'd), then edge/dst-block-parallel
    message passing (3 dst blocks of 128 nodes per core), AllGather.
  - Message passing is expressed as PE matmuls against host-built one-hot
    (edge -> dst) matrices, with exp(e) folded into the one-hot and
    1/denom folded into the PSUM->SBUF epilogue.  Row gathers h[src] use
    the SWDGE dma_gather primitive against DRAM-resident feature tables.

All FLOPs run on device in bf16 with fp32 accumulation.  Host-side work is
restricted to layout (transposes / shards) and integer index preprocessing.
"""
import os
import numpy as np
import ml_dtypes

import concourse.bass as bass
import concourse.bacc as bacc
import concourse.mybir as mybir
import concourse.tile as tile
from concourse import bass_utils

P = 128
NCORES = 8
NU, NB, N, H, HEADS, B = 1024, 2048, 3072, 768, 8, 1024
NIMG = 3
HB = H // P            # 6 channel blocks of 128
NBLK = N // P          # 24 node blocks
BSH = B // NCORES      # 128 batch rows per core
F4 = 4 * H             # 3072 fusion input features
F2 = 2 * H             # 1536
ARW = 896              # AllReduce row width (768 h2 + 2 s2 + pad to 256B-multiple)

BF16 = mybir.dt.bfloat16
F32 = mybir.dt.float32
I16 = mybir.dt.int16
AF = mybir.ActivationFunctionType
ALU = mybir.AluOpType

_nbf = ml_dtypes.bfloat16


def _wrap_idx(idx):
    """[n] -> [128, n/16] int16; index i at (i%16, i//16), replicated to all
    8 gpsimd groups of 16 partitions."""
    idx = np.asarray(idx)
    n = idx.shape[0]
    assert n % 16 == 0
    a = np.zeros((128, n // 16), dtype=np.int16)
    cols = np.arange(n) // 16
    rows = np.arange(n) % 16
    for g in range(8):
        a[rows + 16 * g, cols] = idx.astype(np.int16)
    return a


def _build_blocks(src_s, dst_s, dstblks, nblk_force=None):
    """Edges pre-sorted by dst.  Returns per-dstblk one-hot M [128,nblk,128],
    concatenated padded src/dst index lists, and nblk per dstblk."""
    Ms, srcpad, dstpad, nblks = [], [], [], []
    for d in dstblks:
        sel = (dst_s // P) == d
        sd, dd = src_s[sel], dst_s[sel] - P * d
        n = len(sd)
        nblk = max(1, -(-n // P))
        if nblk_force is not None:
            assert nblk <= nblk_force, (n, nblk_force)
            nblk = nblk_force
        npad = nblk * P
        sp = np.zeros(npad, np.int64)
        sp[:n] = sd
        dp = np.zeros(npad, np.int64)
        dp[:n] = dd + P * d
        M = np.zeros((P, nblk, P), np.float32)
        j = np.arange(n)
        M[j % P, j // P, dd] = 1.0
        Ms.append(M)
        srcpad.append(sp)
        dstpad.append(dp)
        nblks.append(nblk)
    return (np.concatenate(Ms, axis=1),
            np.concatenate(srcpad), np.concatenate(dstpad), nblks)


def host_prep(inputs):
    inp = {k: np.ascontiguousarray(np.asarray(v)) for k, v in inputs.items()}
    user_idx = inp["user_idx"].astype(np.int64)
    business_idx = inp["business_idx"].astype(np.int64)
    ei = inp["edge_index"].astype(np.int64)

    jl = np.full(NB, -1, np.int64)
    jl[business_idx - NU] = np.arange(B)
    bmask = (jl >= 0).astype(np.float32)
    jl = np.where(jl < 0, 0, jl)
    u_mask = np.zeros(NU, np.float32)
    u_mask[user_idx] = 1.0

    src = np.concatenate([ei[0], np.arange(N)])
    dst = np.concatenate([ei[1], np.arange(N)])
    order = np.argsort(dst, kind="stable")
    src_s, dst_s = src[order], dst[order]

    M1, src1, dst1, nblk1 = _build_blocks(src_s, dst_s, range(NBLK))
    T1 = sum(nblk1)

    # layer 2: uniform block count across cores so the SPMD program matches
    nblk2u = 0
    for d in range(NBLK):
        n = int(np.sum((dst_s // P) == d))
        nblk2u = max(nblk2u, -(-n // P))
    l2 = []
    for k in range(NCORES):
        M2, src2, dst2, nblk2 = _build_blocks(
            src_s, dst_s, range(3 * k, 3 * k + 3), nblk_force=nblk2u)
        l2.append(dict(M2=M2, src2=src2, dst2=dst2))
    T2 = 3 * nblk2u

    pr = dict(
        T1=T1, nblk1=nblk1, T2=T2, nblk2u=nblk2u,
        M1=M1.astype(_nbf),
        src1w=_wrap_idx(src1), dst1w=_wrap_idx(dst1),
        jlw=_wrap_idx(jl),
        u_mask_b=np.broadcast_to(u_mask.astype(_nbf), (P, NU)).copy(),
        bm025_b=np.broadcast_to((0.25 * bmask).astype(_nbf), (P, NB)).copy(),
        ident=np.eye(P, dtype=_nbf),
        l2=[dict(M2=c["M2"].astype(_nbf), src2w=_wrap_idx(c["src2"]),
                 dst2w=_wrap_idx(c["dst2"])) for c in l2],
        uiw=[_wrap_idx(user_idx[k * BSH:(k + 1) * BSH]) for k in range(NCORES)],
        biw=[_wrap_idx(business_idx[k * BSH:(k + 1) * BSH]) for k in range(NCORES)],
        has_b1=bool(np.any(inp["b1"] != 0)),
        has_b2=bool(np.any(inp["b2"] != 0)),
        bf3_val=float(inp["bf3"][0]),
        inp=inp,
    )
    return pr


def build_program(pr, debug=False):
    T1, nblk1, T2, nblk2u = pr["T1"], pr["nblk1"], pr["T2"], pr["nblk2u"]
    has_b1, has_b2 = pr["has_b1"], pr["has_b2"]

    nc = bacc.Bacc("TRN2", target_bir_lowering=False, debug=False,
                   num_devices=NCORES)
    D = nc.dram_tensor

    # ---- inputs ----
    t_text = D("text_clsT", [H, BSH], F32, kind="ExternalInput")
    t_img = D("imgT", [NIMG, H, BSH], F32, kind="ExternalInput")
    t_bizf = D("bizfT", [3, BSH], F32, kind="ExternalInput")
    t_wtext = D("W_text", [H, H], F32, kind="ExternalInput")
    t_wimg = D("W_img", [H, H], F32, kind="ExternalInput")
    t_wbf = D("W_bf", [3, H], F32, kind="ExternalInput")
    t_btext = D("b_text", [H], F32, kind="ExternalInput")
    t_bimg = D("b_img", [H], F32, kind="ExternalInput")
    t_bbf = D("b_bf", [H], F32, kind="ExternalInput")
    t_usertT = D("user_tableT", [H, NU], F32, kind="ExternalInput")
    t_biztT = D("biz_tableT", [H, NB], F32, kind="ExternalInput")
    t_w1 = D("W1k", [H, H], F32, kind="ExternalInput")
    t_w2 = D("W2k", [H, H], F32, kind="ExternalInput")
    t_a1 = D("a1k", [H, 2], F32, kind="ExternalInput")
    t_w1T = D("W1kT", [H, H], F32, kind="ExternalInput")
    t_w2T = D("W2kT", [H, H], F32, kind="ExternalInput")
    t_a2 = D("a2", [H, 2], F32, kind="ExternalInput")
    t_wf1 = D("Wf1", [F4, F2], F32, kind="ExternalInput")
    t_wf2 = D("Wf2", [F2, H], F32, kind="ExternalInput")
    t_wf3 = D("Wf3", [H, 1], F32, kind="ExternalInput")
    t_bf1 = D("bf1", [F2], F32, kind="ExternalInput")
    t_bf2 = D("bf2", [H], F32, kind="ExternalInput")
    t_m1 = D("M1", [P, T1, P], BF16, kind="ExternalInput")
    t_s1w = D("src1w", [P, T1 * 8], I16, kind="ExternalInput")
    t_d1w = D("dst1w", [P, T1 * 8], I16, kind="ExternalInput")
    t_m2 = D("M2", [P, T2, P], BF16, kind="ExternalInput")
    t_s2w = D("src2w", [P, T2 * 8], I16, kind="ExternalInput")
    t_d2w = D("dst2w", [P, T2 * 8], I16, kind="ExternalInput")
    t_jlw = D("jlw", [P, NB // 16], I16, kind="ExternalInput")
    t_uiw = D("uiw", [P, BSH // 16], I16, kind="ExternalInput")
    t_biw = D("biw", [P, BSH // 16], I16, kind="ExternalInput")
    t_um = D("u_mask_b", [P, NU], BF16, kind="ExternalInput")
    t_bm = D("bm025_b", [P, NB], BF16, kind="ExternalInput")
    t_id = D("ident", [P, P], BF16, kind="ExternalInput")
    if has_b1:
        t_b1b = D("b1_b", [P, H], F32, kind="ExternalInput")
    if has_b2:
        t_b2b = D("b2_b", [P, H], F32, kind="ExternalInput")

    t_y = D("y", [P, 1], F32, kind="ExternalOutput")
    dbg = {}
    if debug:
        dbg["x2"] = D("dbg_x2", [P, NBLK, H], F32, kind="ExternalOutput")
        dbg["den"] = D("dbg_den", [P, NBLK], F32, kind="ExternalOutput")
        dbg["xo"] = D("dbg_xo", [N, H], BF16, kind="ExternalOutput")
        dbg["svec"] = D("dbg_svec", [P, NBLK, 2], F32, kind="ExternalOutput")

    rg = [list(range(NCORES))]

    with tile.TileContext(nc) as tc:
        sy = nc.sync
        gp = nc.gpsimd
        ve = nc.vector
        sc = nc.scalar
        te = nc.tensor

        with (tc.tile_pool(name="pp", bufs=1) as pp,
              tc.tile_pool(name="ps_big", bufs=2, space="PSUM") as ps_big,
              tc.tile_pool(name="ps_mid", bufs=2, space="PSUM") as ps_mid,
              tc.tile_pool(name="ps_sml", bufs=2, space="PSUM") as ps_sml,
              tc.tile_pool(name="dram", bufs=1, space="DRAM") as dram):

            # cross-phase persistent tiles
            textT = pp.tile([P, HB, BSH], BF16, tag="textT")
            imgT = pp.tile([P, HB, BSH], BF16, tag="imgT")
            s_ag_in = dram.tile([BSH, H], BF16)
            s_full = dram.tile([B, H], BF16)
            fat_dram = dram.tile([N, 64], F32)
            h_dram = dram.tile([N, H], BF16)
            x2_dram = dram.tile([N, H], BF16)
            ar_in = dram.tile([N, ARW], BF16)
            ar_out = dram.tile([N, ARW], BF16)
            fat2_dram = dram.tile([N, 64], F32)
            ag_in = dram.tile([3 * P, H], BF16)
            xo_dram = dram.tile([N, H], BF16)

            # ====== phase 0: encoders (transposed, batch shard) ======
            with (tc.tile_pool(name="ep", bufs=1) as ep,
                  tc.tile_pool(name="ep2", bufs=2) as ep2):
                wtext = ep.tile([P, HB, H], BF16, tag="wtext")
                gp.dma_start(wtext[:], t_wtext[:].rearrange("(a p) c -> p a c", p=P))
                wimg = ep.tile([P, HB, H], BF16, tag="wimg")
                gp.dma_start(wimg[:], t_wimg[:].rearrange("(a p) c -> p a c", p=P))
                wbf = ep.tile([3, H], BF16, tag="wbf")
                gp.dma_start(wbf[:], t_wbf[:])
                btext = ep.tile([P, HB], F32, tag="btext")
                sy.dma_start(btext[:], t_btext[:].rearrange("(a p) -> p a", p=P))
                bimg = ep.tile([P, HB], F32, tag="bimg")
                sy.dma_start(bimg[:], t_bimg[:].rearrange("(a p) -> p a", p=P))
                bbf = ep.tile([P, HB], F32, tag="bbf")
                sy.dma_start(bbf[:], t_bbf[:].rearrange("(a p) -> p a", p=P))

                tct = ep.tile([P, HB, BSH], BF16, tag="tct")
                gp.dma_start(tct[:], t_text[:].rearrange("(a p) b -> p a b", p=P))
                img0 = ep2.tile([P, HB, BSH], BF16, tag="imgl")
                gp.dma_start(img0[:], t_img[0].rearrange("(a p) b -> p a b", p=P))
                img1 = ep2.tile([P, HB, BSH], BF16, tag="imgl")
                gp.dma_start(img1[:], t_img[1].rearrange("(a p) b -> p a b", p=P))
                img2 = ep.tile([P, HB, BSH], BF16, tag="imgl3")
                gp.dma_start(img2[:], t_img[2].rearrange("(a p) b -> p a b", p=P))
                imgsum = ep.tile([P, HB, BSH], BF16, tag="imgsum")
                ve.tensor_tensor(imgsum[:], img0[:], img1[:], op=ALU.add)
                ve.tensor_tensor(imgsum[:], imgsum[:], img2[:], op=ALU.add)
                bizf = ep.tile([3, BSH], BF16, tag="bizf")
                gp.dma_start(bizf[:], t_bizf[:])

                sT = ep.tile([P, HB, BSH], BF16, tag="sT")
                for co in range(HB):
                    pt = ps_sml.tile([P, BSH], F32, tag="enc")
                    for ci in range(HB):
                        te.matmul(pt[:], wtext[:, ci, co * P:(co + 1) * P],
                                  tct[:, ci, :], start=(ci == 0),
                                  stop=(ci == HB - 1))
                    ve.tensor_scalar(textT[:, co, :], pt[:], btext[:, co:co + 1],
                                     None, ALU.add)
                    pt2 = ps_sml.tile([P, BSH], F32, tag="enc")
                    for ci in range(HB):
                        te.matmul(pt2[:], wimg[:, ci, co * P:(co + 1) * P],
                                  imgsum[:, ci, :], start=(ci == 0),
                                  stop=(ci == HB - 1))
                    ve.tensor_scalar(imgT[:, co, :], pt2[:], 1.0 / 3.0,
                                     bimg[:, co:co + 1], ALU.mult, ALU.add)
                    pt3 = ps_sml.tile([P, BSH], F32, tag="enc")
                    te.matmul(pt3[:], wbf[:, co * P:(co + 1) * P], bizf[:],
                              start=True, stop=True)
                    ve.tensor_scalar(sT[:, co, :], pt3[:], bbf[:, co:co + 1],
                                     None, ALU.add)
                    ve.tensor_tensor(sT[:, co, :], sT[:, co, :], textT[:, co, :],
                                     op=ALU.add)
                    ve.tensor_tensor(sT[:, co, :], sT[:, co, :], imgT[:, co, :],
                                     op=ALU.add)

                # s row-major shard -> DRAM -> AllGather
                ident = ep.tile([P, P], BF16, tag="ident")
                sy.dma_start(ident[:], t_id[:])
                srow = ep.tile([P, H], BF16, tag="srow")
                for ci in range(HB):
                    ptt = ps_sml.tile([P, P], BF16, tag="enc")
                    te.transpose(ptt[:], sT[:, ci, :], ident[:])
                    ve.tensor_copy(srow[:, ci * P:(ci + 1) * P], ptt[:])
                gp.dma_start(s_ag_in[:], srow[:])
            gp.collective_compute("AllGather", ALU.bypass, replica_groups=rg,
                                  ins=[s_ag_in.opt()], outs=[s_full.opt()])

            # ====== phase 1: build x^T + layer-1 local matmuls ======
            xT = pp.tile([P, HB, N], BF16, tag="xT")
            with tc.tile_pool(name="xb", bufs=1) as xp:
                jlidx = xp.tile([P, NB // 16], I16, tag="jlidx")
                sy.dma_start(jlidx[:], t_jlw[:])
                umask = xp.tile([P, NU], BF16, tag="umask")
                sy.dma_start(umask[:], t_um[:])
                bmask = xp.tile([P, NB], BF16, tag="bmask")
                sy.dma_start(bmask[:], t_bm[:])

                ut = xp.tile([P, HB, NU], BF16, tag="ut")
                gp.dma_start(ut[:], t_usertT[:].rearrange("(a p) n -> p a n", p=P))
                for c in range(HB):
                    ve.tensor_tensor(xT[:, c, 0:NU], ut[:, c, :], umask[:],
                                     op=ALU.mult)
                sg = xp.tile([P, HB, NB], BF16, tag="sgath")
                gp.dma_gather(sg[:], s_full[:], jlidx[:], num_idxs=NB,
                              num_idxs_reg=NB, elem_size=H, transpose=True, single_packet=False)
                bt = xp.tile([P, HB, NB], BF16, tag="bt")
                gp.dma_start(bt[:], t_biztT[:].rearrange("(a p) n -> p a n", p=P))
                for c in range(HB):
                    ve.tensor_tensor(sg[:, c, :], sg[:, c, :], bt[:, c, :],
                                     op=ALU.add)
                    ve.tensor_tensor(xT[:, c, NU:N], sg[:, c, :], bmask[:],
                                     op=ALU.mult)

            # ====== layer 1 ======
            with (tc.tile_pool(name="l1", bufs=1) as l1p,
                  tc.tile_pool(name="l1d", bufs=2) as l1d,
                  tc.tile_pool(name="l1t", bufs=3) as l1t):
                w1 = l1p.tile([P, HB, H], BF16, tag="w1")
                gp.dma_start(w1[:], t_w1[:].rearrange("(a p) c -> p a c", p=P))
                a1 = l1p.tile([P, HB, 2], BF16, tag="a1")
                gp.dma_start(a1[:], t_a1[:].rearrange("(a p) c -> p a c", p=P))
                w1T = l1p.tile([P, HB, H], BF16, tag="w1T")
                gp.dma_start(w1T[:], t_w1T[:].rearrange("(a p) c -> p a c", p=P))
                ws1 = l1p.tile([P, HB, 2], BF16, tag="ws1")
                for f in range(HB):
                    pw = ps_sml.tile([P, 2], F32, tag="vec")
                    for co in range(HB):
                        te.matmul(pw[:], w1T[:, co, f * P:(f + 1) * P],
                                  a1[:, co, :], start=(co == 0),
                                  stop=(co == HB - 1))
                    ve.tensor_copy(ws1[:, f, :], pw[:])

                svec = l1p.tile([P, NBLK, 2], F32, tag="svec")
                for nb in range(NBLK):
                    pv = ps_sml.tile([P, 2], F32, tag="vec")
                    for ci in range(HB):
                        te.matmul(pv[:], xT[:, ci, nb * P:(nb + 1) * P],
                                  ws1[:, ci, :], start=(ci == 0),
                                  stop=(ci == HB - 1))
                    ve.tensor_copy(svec[:, nb, :], pv[:])
                if debug:
                    sy.dma_start(dbg["svec"][:], svec[:])

                fat_sb = l1p.tile([P, NBLK, 64], F32, tag="fat_sb")
                ve.memset(fat_sb[:], 0.0)
                ve.tensor_copy(fat_sb[:, :, 0:2], svec[:])
                gp.dma_start(fat_dram[:].rearrange("(a p) c -> p a c", p=P),
                             fat_sb[:])

                # h = x @ W1_k  (row-major), streamed to DRAM
                for nb in range(NBLK):
                    ph1 = ps_big.tile([P, 512], F32, tag="big")
                    ph2 = ps_mid.tile([P, 256], F32, tag="mid")
                    for ci in range(HB):
                        te.matmul(ph1[:], xT[:, ci, nb * P:(nb + 1) * P],
                                  w1[:, ci, 0:512], start=(ci == 0),
                                  stop=(ci == HB - 1))
                    for ci in range(HB):
                        te.matmul(ph2[:], xT[:, ci, nb * P:(nb + 1) * P],
                                  w1[:, ci, 512:H], start=(ci == 0),
                                  stop=(ci == HB - 1))
                    hst = l1t.tile([P, H], BF16, tag="hst")
                    ve.tensor_copy(hst[:, 0:512], ph1[:])
                    ve.tensor_copy(hst[:, 512:H], ph2[:])
                    sy.dma_start(h_dram[nb * P:(nb + 1) * P, :], hst[:])

                # --- edge phase ---
                s1idx = l1p.tile([P, T1 * 8], I16, tag="s1idx")
                sy.dma_start(s1idx[:], t_s1w[:])
                d1idx = l1p.tile([P, T1 * 8], I16, tag="d1idx")
                sy.dma_start(d1idx[:], t_d1w[:])

                ee = l1p.tile([P, T1], F32, tag="ee")
                eebf = l1p.tile([P, T1], BF16, tag="eebf")
                off1 = np.concatenate([[0], np.cumsum(nblk1)]).astype(int)
                groups = [(0, 6), (6, 12), (12, 18), (18, 24)]
                for g0, g1 in groups:
                    o0, o1 = int(off1[g0]), int(off1[g1])
                    cnt = o1 - o0
                    gs = l1d.tile([P, cnt, 64], F32, tag="fatg")
                    gp.dma_gather(gs[:], fat_dram[:], s1idx[:, o0 * 8:o1 * 8],
                                  num_idxs=cnt * P, num_idxs_reg=cnt * P,
                                  elem_size=64, single_packet=False)
                    gd = l1d.tile([P, cnt, 64], F32, tag="fatg2")
                    gp.dma_gather(gd[:], fat_dram[:], d1idx[:, o0 * 8:o1 * 8],
                                  num_idxs=cnt * P, num_idxs_reg=cnt * P,
                                  elem_size=64, single_packet=False)
                    # e = s_src[src] + s_dst[dst]
                    ve.tensor_tensor(ee[:, o0:o1], gs[:, :, 0], gd[:, :, 1],
                                     op=ALU.add)
                et = l1p.tile([P, T1], F32, tag="et")
                ve.tensor_scalar(et[:], ee[:], 0.2, None, ALU.mult)
                ve.tensor_tensor(ee[:], ee[:], et[:], op=ALU.max)
                sc.activation(ee[:], ee[:], AF.Exp)
                ve.tensor_copy(eebf[:], ee[:])

                den = l1p.tile([P, NBLK], F32, tag="den")
                recip = l1p.tile([P, NBLK], F32, tag="recip")
                if has_b1:
                    b1b = l1p.tile([P, H], F32, tag="b1b")
                    sy.dma_start(b1b[:], t_b1b[:])

                for d in range(NBLK):
                    nblk = nblk1[d]
                    o = int(off1[d])
                    m1 = l1t.tile([P, nblk, P], BF16, tag="m1")
                    sy.dma_start(m1[:], t_m1[:, o:o + nblk, :])
                    pa = ps_sml.tile([P, 2], F32, tag="vec")
                    for b in range(nblk):
                        te.matmul(pa[:, 0:1], m1[:, b, :],
                                  eebf[:, o + b:o + b + 1],
                                  start=(b == 0), stop=(b == nblk - 1))
                    ve.tensor_scalar(den[:, d:d + 1], pa[:, 0:1], 1e-16, None,
                                     ALU.add)
                    ve.reciprocal(recip[:, d:d + 1], den[:, d:d + 1])
                    mbe = l1d.tile([P, nblk, P], BF16, tag="mbe")
                    for b in range(nblk):
                        ve.tensor_scalar(mbe[:, b, :], m1[:, b, :],
                                         ee[:, o + b:o + b + 1], None, ALU.mult)
                    gh = l1d.tile([P, nblk, H], BF16, tag="gh")
                    gp.dma_gather(gh[:], h_dram[:],
                                  s1idx[:, o * 8:(o + nblk) * 8],
                                  num_idxs=nblk * P, num_idxs_reg=nblk * P,
                                  elem_size=H, single_packet=False)
                    pb1 = ps_big.tile([P, 512], F32, tag="big")
                    pb2 = ps_mid.tile([P, 256], F32, tag="mid")
                    for b in range(nblk):
                        te.matmul(pb1[:], mbe[:, b, :], gh[:, b, 0:512],
                                  start=(b == 0), stop=(b == nblk - 1))
                    for b in range(nblk):
                        te.matmul(pb2[:], mbe[:, b, :], gh[:, b, 512:H],
                                  start=(b == 0), stop=(b == nblk - 1))
                    x2st = l1t.tile([P, H], BF16, tag="hst")
                    if has_b1:
                        tmp = l1t.tile([P, H], F32, tag="tmpb")
                        ve.tensor_scalar(tmp[:, 0:512], pb1[:],
                                         recip[:, d:d + 1], None, ALU.mult)
                        ve.tensor_scalar(tmp[:, 512:H], pb2[:],
                                         recip[:, d:d + 1], None, ALU.mult)
                        ve.tensor_tensor(tmp[:], tmp[:], b1b[:], op=ALU.add)
                        ve.tensor_scalar(x2st[:], tmp[:], 0.0, None, ALU.max)
                    else:
                        ve.tensor_scalar(x2st[:, 0:512], pb1[:],
                                         recip[:, d:d + 1], 0.0, ALU.mult,
                                         ALU.max)
                        ve.tensor_scalar(x2st[:, 512:H], pb2[:],
                                         recip[:, d:d + 1], 0.0, ALU.mult,
                                         ALU.max)
                    sy.dma_start(x2_dram[d * P:(d + 1) * P, :], x2st[:])
                    if debug:
                        dx = l1t.tile([P, H], F32, tag="dbgx")
                        ve.tensor_copy(dx[:], x2st[:])
                        sy.dma_start(dbg["x2"][:, d, :], dx[:])
                if debug:
                    sy.dma_start(dbg["den"][:], den[:])

            # ====== layer 2 ======
            x2T = pp.tile([P, HB, N], BF16, tag="xT")
            with (tc.tile_pool(name="l2", bufs=1) as l2p,
                  tc.tile_pool(name="l2d", bufs=2) as l2d,
                  tc.tile_pool(name="l2t", bufs=3) as l2t):
                for c in range(HB):
                    sy.dma_start_transpose(x2T[:, c, :],
                                           x2_dram[:, c * P:(c + 1) * P])
                w2 = l2p.tile([P, HB, H], BF16, tag="w2")
                gp.dma_start(w2[:], t_w2[:].rearrange("(a p) c -> p a c", p=P))
                a2 = l2p.tile([P, HB, 2], BF16, tag="a2")
                gp.dma_start(a2[:], t_a2[:].rearrange("(a p) c -> p a c", p=P))
                w2T = l2p.tile([P, HB, H], BF16, tag="w2T")
                gp.dma_start(w2T[:], t_w2T[:].rearrange("(a p) c -> p a c", p=P))
                ws2 = l2p.tile([P, HB, 2], BF16, tag="ws2")
                for f in range(HB):
                    pw = ps_sml.tile([P, 2], F32, tag="vec")
                    for co in range(HB):
                        te.matmul(pw[:], w2T[:, co, f * P:(f + 1) * P],
                                  a2[:, co, :], start=(co == 0),
                                  stop=(co == HB - 1))
                    ve.tensor_copy(ws2[:, f, :], pw[:])

                for nb in range(NBLK):
                    ph1 = ps_big.tile([P, 512], F32, tag="big")
                    ph2 = ps_mid.tile([P, 256], F32, tag="mid")
                    pv = ps_sml.tile([P, 2], F32, tag="vec")
                    for ci in range(HB):
                        te.matmul(ph1[:], x2T[:, ci, nb * P:(nb + 1) * P],
                                  w2[:, ci, 0:512], start=(ci == 0),
                                  stop=(ci == HB - 1))
                    for ci in range(HB):
                        te.matmul(ph2[:], x2T[:, ci, nb * P:(nb + 1) * P],
                                  w2[:, ci, 512:H], start=(ci == 0),
                                  stop=(ci == HB - 1))
                    for ci in range(HB):
                        te.matmul(pv[:], x2T[:, ci, nb * P:(nb + 1) * P],
                                  ws2[:, ci, :], start=(ci == 0),
                                  stop=(ci == HB - 1))
                    ast = l2t.tile([P, ARW], BF16, tag="ast")
                    ve.memset(ast[:, 770:ARW], 0.0)
                    ve.tensor_copy(ast[:, 0:512], ph1[:])
                    ve.tensor_copy(ast[:, 512:H], ph2[:])
                    ve.tensor_copy(ast[:, H:770], pv[:])
                    sy.dma_start(ar_in[nb * P:(nb + 1) * P, :], ast[:])
                gp.collective_compute("AllReduce", ALU.add, replica_groups=rg,
                                      ins=[ar_in.opt()], outs=[ar_out.opt()])

                # --- layer 2 edge phase (3 local dstblks) ---
                svec2 = l2p.tile([P, NBLK, 2], F32, tag="svec2")
                s2bf = l2d.tile([P, NBLK, 2], BF16, tag="s2bf")
                sy.dma_start(s2bf[:],
                             ar_out[:, H:770].rearrange("(a p) c -> p a c", p=P))
                ve.tensor_copy(svec2[:], s2bf[:])
                fat2_sb = l2p.tile([P, NBLK, 64], F32, tag="fat_sb")
                ve.memset(fat2_sb[:], 0.0)
                ve.tensor_copy(fat2_sb[:, :, 0:2], svec2[:])
                gp.dma_start(fat2_dram[:].rearrange("(a p) c -> p a c", p=P),
                             fat2_sb[:])

                s2idx = l2p.tile([P, T2 * 8], I16, tag="s2idx")
                sy.dma_start(s2idx[:], t_s2w[:])
                d2idx = l2p.tile([P, T2 * 8], I16, tag="d2idx")
                sy.dma_start(d2idx[:], t_d2w[:])

                ee2 = l2p.tile([P, T2], F32, tag="ee2")
                gs2 = l2d.tile([P, T2, 64], F32, tag="fatg")
                gp.dma_gather(gs2[:], fat2_dram[:], s2idx[:], num_idxs=T2 * P,
                              num_idxs_reg=T2 * P, elem_size=64, single_packet=False)
                gd2 = l2d.tile([P, T2, 64], F32, tag="fatg2")
                gp.dma_gather(gd2[:], fat2_dram[:], d2idx[:], num_idxs=T2 * P,
                              num_idxs_reg=T2 * P, elem_size=64, single_packet=False)
                ve.tensor_tensor(ee2[:], gs2[:, :, 0], gd2[:, :, 1], op=ALU.add)
                et2 = l2p.tile([P, T2], F32, tag="et2")
                ve.tensor_scalar(et2[:], ee2[:], 0.2, None, ALU.mult)
                ve.tensor_tensor(ee2[:], ee2[:], et2[:], op=ALU.max)
                sc.activation(ee2[:], ee2[:], AF.Exp)
                ee2bf = l2p.tile([P, T2], BF16, tag="ee2bf")
                ve.tensor_copy(ee2bf[:], ee2[:])

                den2 = l2p.tile([P, 3], F32, tag="den2")
                recip2 = l2p.tile([P, 3], F32, tag="recip2")
                if has_b2:
                    b2b = l2p.tile([P, H], F32, tag="b2b")
                    sy.dma_start(b2b[:], t_b2b[:])

                for dl in range(3):
                    o = dl * nblk2u
                    m2 = l2t.tile([P, nblk2u, P], BF16, tag="m1")
                    sy.dma_start(m2[:], t_m2[:, o:o + nblk2u, :])
                    pa = ps_sml.tile([P, 2], F32, tag="vec")
                    for b in range(nblk2u):
                        te.matmul(pa[:, 0:1], m2[:, b, :],
                                  ee2bf[:, o + b:o + b + 1],
                                  start=(b == 0), stop=(b == nblk2u - 1))
                    ve.tensor_scalar(den2[:, dl:dl + 1], pa[:, 0:1], 1e-16,
                                     None, ALU.add)
                    ve.reciprocal(recip2[:, dl:dl + 1], den2[:, dl:dl + 1])
                    mbe = l2d.tile([P, nblk2u, P], BF16, tag="mbe")
                    for b in range(nblk2u):
                        ve.tensor_scalar(mbe[:, b, :], m2[:, b, :],
                                         ee2[:, o + b:o + b + 1], None,
                                         ALU.mult)
                    gh = l2d.tile([P, nblk2u, H], BF16, tag="gh")
                    gp.dma_gather(gh[:], ar_out[:, 0:H],
                                  s2idx[:, o * 8:(o + nblk2u) * 8],
                                  num_idxs=nblk2u * P, num_idxs_reg=nblk2u * P,
                                  elem_size=H, elem_step=ARW, single_packet=False)
                    pb1 = ps_big.tile([P, 512], F32, tag="big")
                    pb2 = ps_mid.tile([P, 256], F32, tag="mid")
                    for b in range(nblk2u):
                        te.matmul(pb1[:], mbe[:, b, :], gh[:, b, 0:512],
                                  start=(b == 0), stop=(b == nblk2u - 1))
                    for b in range(nblk2u):
                        te.matmul(pb2[:], mbe[:, b, :], gh[:, b, 512:H],
                                  start=(b == 0), stop=(b == nblk2u - 1))
                    xost = l2t.tile([P, H], BF16, tag="hst")
                    if has_b2:
                        tmp = l2t.tile([P, H], F32, tag="tmpb")
                        ve.tensor_scalar(tmp[:, 0:512], pb1[:],
                                         recip2[:, dl:dl + 1], None, ALU.mult)
                        ve.tensor_scalar(tmp[:, 512:H], pb2[:],
                                         recip2[:, dl:dl + 1], None, ALU.mult)
                        ve.tensor_tensor(xost[:], tmp[:], b2b[:], op=ALU.add)
                    else:
                        ve.tensor_scalar(xost[:, 0:512], pb1[:],
                                         recip2[:, dl:dl + 1], None, ALU.mult)
                        ve.tensor_scalar(xost[:, 512:H], pb2[:],
                                         recip2[:, dl:dl + 1], None, ALU.mult)
                    sy.dma_start(ag_in[dl * P:(dl + 1) * P, :], xost[:])
                gp.collective_compute("AllGather", ALU.bypass, replica_groups=rg,
                                      ins=[ag_in.opt()], outs=[xo_dram.opt()])
                if debug:
                    xodbg = l2d.tile([P, NBLK, H], BF16, tag="xodbg")
                    gp.dma_start(xodbg[:],
                                 xo_dram[:].rearrange("(a p) c -> p a c", p=P))
                    gp.dma_start(dbg["xo"][:].rearrange("(a p) c -> p a c", p=P),
                                 xodbg[:])

            # ====== fusion MLP (batch shard) ======
            with (tc.tile_pool(name="fu", bufs=1) as fp,
                  tc.tile_pool(name="fud", bufs=2) as fd):
                uidx = fp.tile([P, BSH // 16], I16, tag="uidx")
                sy.dma_start(uidx[:], t_uiw[:])
                bidx = fp.tile([P, BSH // 16], I16, tag="bidx")
                sy.dma_start(bidx[:], t_biw[:])
                xuT = fp.tile([P, HB, BSH], BF16, tag="xuT")
                gp.dma_gather(xuT[:], xo_dram[:], uidx[:], num_idxs=BSH,
                              num_idxs_reg=BSH, elem_size=H, transpose=True, single_packet=False)
                xbT = fp.tile([P, HB, BSH], BF16, tag="xbT")
                gp.dma_gather(xbT[:], xo_dram[:], bidx[:], num_idxs=BSH,
                              num_idxs_reg=BSH, elem_size=H, transpose=True, single_packet=False)

                bf1 = fp.tile([P, F2 // P], F32, tag="bf1")
                sy.dma_start(bf1[:], t_bf1[:].rearrange("(a p) -> p a", p=P))
                bf2 = fp.tile([P, HB], F32, tag="bf2")
                sy.dma_start(bf2[:], t_bf2[:].rearrange("(a p) -> p a", p=P))

                cat_tiles = [xuT, xbT, textT, imgT]
                h1fT = fp.tile([P, F2 // P, BSH], BF16, tag="h1fT")
                for ob in range(F2 // P):
                    wf1 = fd.tile([P, F4 // P, P], BF16, tag="wf1")
                    gp.dma_start(
                        wf1[:],
                        t_wf1[:, ob * P:(ob + 1) * P].rearrange(
                            "(a p) c -> p a c", p=P))
                    pf = ps_sml.tile([P, BSH], F32, tag="enc")
                    for fb in range(F4 // P):
                        rhs = cat_tiles[fb // HB][:, fb % HB, :]
                        te.matmul(pf[:], wf1[:, fb, :], rhs, start=(fb == 0),
                                  stop=(fb == F4 // P - 1))
                    ve.tensor_scalar(h1fT[:, ob, :], pf[:], bf1[:, ob:ob + 1],
                                     0.0, ALU.add, ALU.max)

                h2fT = fp.tile([P, HB, BSH], BF16, tag="h2fT")
                for ob in range(HB):
                    wf2 = fd.tile([P, F2 // P, P], BF16, tag="wf2")
                    gp.dma_start(
                        wf2[:],
                        t_wf2[:, ob * P:(ob + 1) * P].rearrange(
                            "(a p) c -> p a c", p=P))
                    pf = ps_sml.tile([P, BSH], F32, tag="enc")
                    for fb in range(F2 // P):
                        te.matmul(pf[:], wf2[:, fb, :], h1fT[:, fb, :],
                                  start=(fb == 0), stop=(fb == F2 // P - 1))
                    ve.tensor_scalar(h2fT[:, ob, :], pf[:], bf2[:, ob:ob + 1],
                                     0.0, ALU.add, ALU.max)

                wf3 = fp.tile([P, HB, 1], BF16, tag="wf3")
                gp.dma_start(wf3[:], t_wf3[:].rearrange("(a p) c -> p a c", p=P))
                py = ps_sml.tile([P, 2], F32, tag="vec")
                for c in range(HB):
                    te.matmul(py[:, 0:1], h2fT[:, c, :], wf3[:, c, :],
                              start=(c == 0), stop=(c == HB - 1))
                ysb = fp.tile([P, 1], F32, tag="ysb")
                ve.tensor_scalar(ysb[:], py[:, 0:1], pr["bf3_val"], None,
                                 ALU.add)
                sy.dma_start(t_y[:], ysb[:])

    nc.compile()
    return nc


def make_in_maps(pr):
    inp = pr["inp"]
    f32 = np.float32
    text_clsT = np.ascontiguousarray(inp["text_cls"].T.astype(f32))
    imgT = np.ascontiguousarray(inp["img_cls"].transpose(1, 2, 0).astype(f32))
    bizfT = np.ascontiguousarray(inp["biz_feats"].T.astype(f32))
    usertT = np.ascontiguousarray(inp["user_table"].T.astype(f32))
    biztT = np.ascontiguousarray(inp["biz_table"].T.astype(f32))
    a2 = np.ascontiguousarray(
        np.stack([inp["att_src2"][0], inp["att_dst2"][0]], axis=1).astype(f32))
    in_maps = []
    for k in range(NCORES):
        sl = slice(k * BSH, (k + 1) * BSH)
        m = dict(
            text_clsT=text_clsT[:, sl].copy(),
            imgT=imgT[:, :, sl].copy(),
            bizfT=bizfT[:, sl].copy(),
            W_text=inp["W_text"].astype(f32),
            W_img=inp["W_img"].astype(f32),
            W_bf=inp["W_bf"].astype(f32),
            b_text=inp["b_text"].astype(f32),
            b_img=inp["b_img"].astype(f32),
            b_bf=inp["b_bf"].astype(f32),
            user_tableT=usertT,
            biz_tableT=biztT,
            W1k=np.ascontiguousarray(inp["W1"][:, k * H:(k + 1) * H].astype(f32)),
            W1kT=np.ascontiguousarray(inp["W1"][:, k * H:(k + 1) * H].T.astype(f32)),
            W2kT=np.ascontiguousarray(inp["W2"][k * H:(k + 1) * H, :].T.astype(f32)),
            W2k=np.ascontiguousarray(inp["W2"][k * H:(k + 1) * H, :].astype(f32)),
            a1k=np.ascontiguousarray(
                np.stack([inp["att_src1"][k], inp["att_dst1"][k]],
                         axis=1).astype(f32)),
            a2=a2,
            Wf1=inp["Wf1"].astype(f32),
            Wf2=inp["Wf2"].astype(f32),
            Wf3=inp["Wf3"].astype(f32),
            bf1=inp["bf1"].astype(f32),
            bf2=inp["bf2"].astype(f32),
            M1=pr["M1"],
            src1w=pr["src1w"], dst1w=pr["dst1w"],
            M2=pr["l2"][k]["M2"],
            src2w=pr["l2"][k]["src2w"], dst2w=pr["l2"][k]["dst2w"],
            jlw=pr["jlw"],
            uiw=pr["uiw"][k], biw=pr["biw"][k],
            u_mask_b=pr["u_mask_b"], bm025_b=pr["bm025_b"],
            ident=pr["ident"],
        )
        if pr["has_b1"]:
            m["b1_b"] = np.broadcast_to(
                inp["b1"][k * H:(k + 1) * H].astype(f32), (P, H)).copy()
        if pr["has_b2"]:
            m["b2_b"] = np.broadcast_to(inp["b2"].astype(f32), (P, H)).copy()
        in_maps.append(m)
    return in_maps


def run(inputs, debug=False, want_results=False):
    pr = host_prep(inputs)
    nc = build_program(pr, debug=debug)
    in_maps = make_in_maps(pr)
    res = bass_utils.run_bass_kernel_spmd(
        nc, in_maps, core_ids=list(range(NCORES)), trace=False)
    y = np.concatenate([res.results[k]["y"][:, 0] for k in range(NCORES)])
    if want_results:
        return y.astype(np.float32), res, pr, nc, in_maps
    return y.astype(np.float32)


def kernel(**inputs):
    return run(inputs)



# revision 16
# speedup vs baseline: 7.3552x; 7.3552x over previous
"""Trainium2 Bass kernel for nn_GATRecommender (8 NeuronCores).

Sharding strategy (v2):
  - Encoders: replicated (every core computes s for the whole batch, with a
    per-core batch rotation so each core's local shard sits at columns 0:128).
    No AllGather for s.
  - GAT layer 1 (8 heads): one head per core; node features replicated.
    Edge aggregation via on-device-built weighted one-hot (iota-compare
    against dst-in-block ids x exp(e)), denominators folded into an
    mbe @ ones PE chain.  x2 transposed on-PE straight into SBUF.
  - GAT layer 2: contraction sharded by head; ReduceScatter + AllGather
    replace the AllReduce.  The L2 edge phase is sharded by each core's
    *fusion needs*: core k computes GAT-2 rows exactly for the nodes its
    local batch indexes, so no final AllGather and no fusion gathers --
    fusion rows come from tiny one-hot permutation matmuls out of SBUF.
  - Fusion MLP: data-parallel over the batch (128 rows/core).

All heavy inputs are host-cast to bf16 (layout prep); FLOPs run on device
in bf16 with fp32 accumulation.
"""
import numpy as np
import ml_dtypes

import concourse.bass as bass
import concourse.bacc as bacc
import concourse.mybir as mybir
import concourse.tile as tile
from concourse import bass_utils

P = 128
NCORES = 8
NU, NB, N, H, HEADS, B = 1024, 2048, 3072, 768, 8, 1024
NIMG = 3
HB = H // P            # 6 channel blocks of 128
NBLK = N // P          # 24 node blocks
BSH = B // NCORES      # 128 batch rows per core
BB = B // P            # 8 batch blocks
F4 = 4 * H             # 3072 fusion input features
F2 = 2 * H             # 1536
ARW = 896              # AllGather row width (768 h2 + 2 s2 + pad to 256B)
NSLOT = 256            # per-core L2 output slots (2 blocks of 128)

BF16 = mybir.dt.bfloat16
F32 = mybir.dt.float32
I16 = mybir.dt.int16
AF = mybir.ActivationFunctionType
ALU = mybir.AluOpType

_nbf = ml_dtypes.bfloat16


def _wrap_idx(idx):
    """[n] -> [128, n/16] int16; index i at (i%16, i//16), replicated to all
    8 gpsimd groups of 16 partitions."""
    idx = np.asarray(idx)
    n = idx.shape[0]
    assert n % 16 == 0
    a = np.zeros((128, n // 16), dtype=np.int16)
    cols = np.arange(n) // 16
    rows = np.arange(n) % 16
    for g in range(8):
        a[rows + 16 * g, cols] = idx.astype(np.int16)
    return a


def _edge_blocks(src_sel, dcol_sel, blk_of, n_blocks, nblk_force=None):
    """Edges (already restricted + sorted by block) -> per-block padded lists.

    src_sel: src node id per edge; dcol_sel: dst-slot-in-block per edge;
    blk_of: block index per edge.  Returns (srcpad [T*128], dcolpad [128, T],
    dstpad [T*128] destination node ids for fat-dst gathers, nblks).
    dcol pads are -1 so the on-device compare never matches them.
    """
    srcs, dcols, nblks = [], [], []
    for d in range(n_blocks):
        sel = blk_of == d
        sd, dd = src_sel[sel], dcol_sel[sel]
        n = len(sd)
        nblk = max(1, -(-n // P))
        if nblk_force is not None:
            assert nblk <= nblk_force, (n, nblk_force)
            nblk = nblk_force
        npad = nblk * P
        sp = np.zeros(npad, np.int64)
        sp[:n] = sd
        dc = np.full(npad, -1.0, np.float32)
        dc[:n] = dd
        srcs.append(sp)
        dcols.append(dc)
        nblks.append(nblk)
    srcpad = np.concatenate(srcs)
    dcol_flat = np.concatenate(dcols)
    T = dcol_flat.shape[0] // P
    dcolw = np.zeros((P, T), np.float32)
    j = np.arange(T * P)
    dcolw[j % P, j // P] = dcol_flat
    return srcpad, dcolw, nblks


def host_prep(inputs):
    inp = {k: np.ascontiguousarray(np.asarray(v)) for k, v in inputs.items()}
    user_idx = inp["user_idx"].astype(np.int64)
    business_idx = inp["business_idx"].astype(np.int64)
    ei = inp["edge_index"].astype(np.int64)

    jl = np.full(NB, -1, np.int64)
    jl[business_idx - NU] = np.arange(B)
    bmask = (jl >= 0).astype(np.float32)
    jl = np.where(jl < 0, 0, jl)
    u_mask = np.zeros(NU, np.float32)
    u_mask[user_idx] = 1.0

    src = np.concatenate([ei[0], np.arange(N)])
    dst = np.concatenate([ei[1], np.arange(N)])
    order = np.argsort(dst, kind="stable")
    src_s, dst_s = src[order], dst[order]

    # ---- layer 1: all 24 dst blocks per core ----
    src1, d1col, nblk1 = _edge_blocks(src_s, dst_s % P, dst_s // P, NBLK)
    T1 = sum(nblk1)
    dst1 = np.zeros(T1 * P, np.int64)
    off1 = np.concatenate([[0], np.cumsum(nblk1)]).astype(int)
    for d in range(NBLK):
        sel = (dst_s // P) == d
        n = int(sel.sum())
        dst1[off1[d] * P: off1[d] * P + n] = dst_s[sel]

    # ---- layer 2: fusion-aligned slots per core ----
    l2 = []
    nblk2u = 1
    per_core = []
    for k in range(NCORES):
        uk = user_idx[k * BSH:(k + 1) * BSH]
        bk = business_idx[k * BSH:(k + 1) * BSH]
        nodes_u = np.unique(np.concatenate([uk, bk]))
        assert len(nodes_u) <= NSLOT
        nodes = np.zeros(NSLOT, np.int64)
        nodes[:len(nodes_u)] = nodes_u
        sel = np.isin(dst_s, nodes_u)
        sd2 = src_s[sel]
        slot = np.searchsorted(nodes_u, dst_s[sel])
        per_core.append((uk, bk, nodes_u, nodes, sd2, slot))
        for sb in range(2):
            nsel = int(np.sum(slot // P == sb))
            nblk2u = max(nblk2u, -(-nsel // P))
    T2 = 2 * nblk2u
    for k in range(NCORES):
        uk, bk, nodes_u, nodes, sd2, slot = per_core[k]
        src2, d2col, _ = _edge_blocks(sd2, slot % P, slot // P, 2,
                                      nblk_force=nblk2u)
        # dst node ids (for fat2-dst gather), padded with 0
        dst2 = np.zeros(T2 * P, np.int64)
        for sb in range(2):
            selb = slot // P == sb
            n = int(selb.sum())
            dst2[sb * nblk2u * P: sb * nblk2u * P + n] = nodes_u[slot[selb]]
        PU = np.zeros((P, 2, P), np.float32)
        PB = np.zeros((P, 2, P), np.float32)
        su = np.searchsorted(nodes_u, uk)
        sb_ = np.searchsorted(nodes_u, bk)
        PU[su % P, su // P, np.arange(BSH)] = 1.0
        PB[sb_ % P, sb_ // P, np.arange(BSH)] = 1.0
        jlk = (jl - k * BSH) % B
        l2.append(dict(
            src2w=_wrap_idx(src2), dst2w=_wrap_idx(dst2), d2col=d2col,
            PU=PU.astype(_nbf), PB=PB.astype(_nbf), jlw=_wrap_idx(jlk)))

    irow = np.broadcast_to(np.arange(P, dtype=np.float32), (P, P))

    pr = dict(
        T1=T1, nblk1=nblk1, T2=T2, nblk2u=nblk2u,
        src1w=_wrap_idx(src1), dst1w=_wrap_idx(dst1), d1col=d1col,
        l2=l2,
        u_mask_b=np.broadcast_to(u_mask.astype(_nbf), (P, NU)).copy(),
        bm025_b=np.broadcast_to((0.25 * bmask).astype(_nbf), (P, NB)).copy(),
        ident=np.eye(P, dtype=_nbf),
        irow=irow.astype(_nbf).copy(),
        has_b1=bool(np.any(inp["b1"] != 0)),
        has_b2=bool(np.any(inp["b2"] != 0)),
        bf3_val=float(inp["bf3"][0]),
        inp=inp,
    )
    return pr


def build_program(pr, debug=False, reps=1):
    T1, nblk1, T2, nblk2u = pr["T1"], pr["nblk1"], pr["T2"], pr["nblk2u"]
    has_b1, has_b2 = pr["has_b1"], pr["has_b2"]

    nc = bacc.Bacc("TRN2", target_bir_lowering=False, debug=False,
                   num_devices=NCORES)
    D = nc.dram_tensor

    # ---- inputs (heavy ones pre-cast to bf16 on host) ----
    t_text = D("text_clsT", [H, B], BF16, kind="ExternalInput")
    t_img = D("imgT", [NIMG, H, B], BF16, kind="ExternalInput")
    t_bizf = D("bizfT", [3, B], BF16, kind="ExternalInput")
    t_wtext = D("W_text", [H, H], BF16, kind="ExternalInput")
    t_wimg = D("W_img", [H, H], BF16, kind="ExternalInput")
    t_wbf = D("W_bf", [3, H], BF16, kind="ExternalInput")
    t_btext = D("b_text", [H], F32, kind="ExternalInput")
    t_bimg = D("b_img", [H], F32, kind="ExternalInput")
    t_bbf = D("b_bf", [H], F32, kind="ExternalInput")
    t_usertT = D("user_tableT", [H, NU], BF16, kind="ExternalInput")
    t_biztT = D("biz_tableT", [H, NB], BF16, kind="ExternalInput")
    t_w1 = D("W1k", [H, H], BF16, kind="ExternalInput")
    t_w1T = D("W1kT", [H, H], BF16, kind="ExternalInput")
    t_a1 = D("a1k", [H, 2], BF16, kind="ExternalInput")
    t_w2 = D("W2k", [H, H], BF16, kind="ExternalInput")
    t_w2T = D("W2kT", [H, H], BF16, kind="ExternalInput")
    t_a2 = D("a2", [H, 2], BF16, kind="ExternalInput")
    t_wf1 = D("Wf1", [F4, F2], BF16, kind="ExternalInput")
    t_wf2 = D("Wf2", [F2, H], BF16, kind="ExternalInput")
    t_wf3 = D("Wf3", [H, 1], BF16, kind="ExternalInput")
    t_bf1 = D("bf1", [F2], F32, kind="ExternalInput")
    t_bf2 = D("bf2", [H], F32, kind="ExternalInput")
    t_s1w = D("src1w", [P, T1 * 8], I16, kind="ExternalInput")
    t_d1w = D("dst1w", [P, T1 * 8], I16, kind="ExternalInput")
    t_d1col = D("d1col", [P, T1], F32, kind="ExternalInput")
    t_s2w = D("src2w", [P, T2 * 8], I16, kind="ExternalInput")
    t_d2w = D("dst2w", [P, T2 * 8], I16, kind="ExternalInput")
    t_d2col = D("d2col", [P, T2], F32, kind="ExternalInput")
    t_pu = D("PU", [P, 2, P], BF16, kind="ExternalInput")
    t_pb = D("PB", [P, 2, P], BF16, kind="ExternalInput")
    t_jlw = D("jlw", [P, NB // 16], I16, kind="ExternalInput")
    t_um = D("u_mask_b", [P, NU], BF16, kind="ExternalInput")
    t_bm = D("bm025_b", [P, NB], BF16, kind="ExternalInput")
    t_id = D("ident", [P, P], BF16, kind="ExternalInput")
    t_ir = D("irow", [P, P], BF16, kind="ExternalInput")
    if has_b1:
        t_b1b = D("b1_b", [P, H], F32, kind="ExternalInput")
    if has_b2:
        t_b2b = D("b2_b", [P, H], F32, kind="ExternalInput")

    t_y = D("y", [P, 1], F32, kind="ExternalOutput")
    dbg = {}
    if debug:
        dbg["sT"] = D("dbg_sT", [P, HB, B], F32, kind="ExternalOutput")
        dbg["xT"] = D("dbg_xT", [P, HB, N], F32, kind="ExternalOutput")
        dbg["svec"] = D("dbg_svec", [P, NBLK, 2], F32, kind="ExternalOutput")
        dbg["ee"] = D("dbg_ee", [P, T1], F32, kind="ExternalOutput")
        dbg["den"] = D("dbg_den", [P, NBLK], F32, kind="ExternalOutput")
        dbg["x2T"] = D("dbg_x2T", [P, HB, N], F32, kind="ExternalOutput")
        dbg["ar"] = D("dbg_ar", [N, ARW], BF16, kind="ExternalOutput")
        dbg["xo"] = D("dbg_xo", [P, 2, H], F32, kind="ExternalOutput")
        dbg["xuT"] = D("dbg_xuT", [P, HB, BSH], F32, kind="ExternalOutput")
        dbg["xbT"] = D("dbg_xbT", [P, HB, BSH], F32, kind="ExternalOutput")

    rg = [list(range(NCORES))]
    off1 = np.concatenate([[0], np.cumsum(nblk1)]).astype(int)
    mx1 = int(max(nblk1))

    with tile.TileContext(nc) as tc:
        sy = nc.sync
        gp = nc.gpsimd
        ve = nc.vector
        sc = nc.scalar
        te = nc.tensor

        with (tc.tile_pool(name="pp", bufs=1) as pp,
              tc.tile_pool(name="ps_big", bufs=2, space="PSUM") as ps_big,
              tc.tile_pool(name="ps_mid", bufs=2, space="PSUM") as ps_mid,
              tc.tile_pool(name="ps_sml", bufs=3, space="PSUM") as ps_sml,
              tc.tile_pool(name="dram", bufs=1, space="DRAM") as dram):

            for _rep in range(reps):
                # persistent tiles (tags shared across reps)
                textT = pp.tile([P, HB, B], BF16, tag="textT")
                imgT = pp.tile([P, HB, B], BF16, tag="imgT")
                ident = pp.tile([P, P], BF16, tag="ident")
                irow = pp.tile([P, P], BF16, tag="irow")
                ones = pp.tile([P, 1], BF16, tag="ones")
                sy.dma_start(ident[:], t_id[:])
                sy.dma_start(irow[:], t_ir[:])
                ve.memset(ones[:], 1.0)

                s_dram = dram.tile([B, H], BF16)
                fat_dram = dram.tile([N, 64], F32)
                h_dram = dram.tile([N, H], BF16)
                ar_in = dram.tile([N, ARW], BF16)
                arrs = dram.tile([N // NCORES, ARW], BF16)
                ar_out = dram.tile([N, ARW], BF16, addr_space="Shared")
                fat2_dram = dram.tile([N, 64], F32)

                # ====== phase E: encoders (replicated, rotated batch) ======
                with (tc.tile_pool(name="ep", bufs=1) as ep,
                      tc.tile_pool(name="ep2", bufs=2) as ep2):
                    wtext = ep.tile([P, HB, H], BF16, tag="wtext")
                    sy.dma_start(wtext[:],
                                 t_wtext[:].rearrange("(a p) c -> p a c", p=P))
                    wimg = ep.tile([P, HB, H], BF16, tag="wimg")
                    sy.dma_start(wimg[:],
                                 t_wimg[:].rearrange("(a p) c -> p a c", p=P))
                    wbf = ep.tile([3, H], BF16, tag="wbf")
                    sy.dma_start(wbf[:], t_wbf[:])
                    btext = ep.tile([P, HB], F32, tag="btext")
                    sy.dma_start(btext[:], t_btext[:].rearrange("(a p) -> p a", p=P))
                    bimg = ep.tile([P, HB], F32, tag="bimg")
                    sy.dma_start(bimg[:], t_bimg[:].rearrange("(a p) -> p a", p=P))
                    bbf = ep.tile([P, HB], F32, tag="bbf")
                    sy.dma_start(bbf[:], t_bbf[:].rearrange("(a p) -> p a", p=P))

                    tct = ep.tile([P, HB, B], BF16, tag="tct")
                    gp.dma_start(tct[:], t_text[:].rearrange("(a p) b -> p a b", p=P))
                    imgsum = ep.tile([P, HB, B], BF16, tag="imgsum")
                    gp.dma_start(imgsum[:],
                                 t_img[0].rearrange("(a p) b -> p a b", p=P))
                    imgl = ep2.tile([P, HB, B], BF16, tag="imgl")
                    gp.dma_start(imgl[:], t_img[1].rearrange("(a p) b -> p a b", p=P))
                    ve.tensor_tensor(imgsum[:], imgsum[:], imgl[:], op=ALU.add)
                    imgl2 = ep2.tile([P, HB, B], BF16, tag="imgl")
                    gp.dma_start(imgl2[:], t_img[2].rearrange("(a p) b -> p a b", p=P))
                    ve.tensor_tensor(imgsum[:], imgsum[:], imgl2[:], op=ALU.add)
                    bizf = ep.tile([3, B], BF16, tag="bizf")
                    sy.dma_start(bizf[:], t_bizf[:])

                    sT = ep.tile([P, HB, B], BF16, tag="sT")
                    for co in range(HB):
                        cs = slice(co * P, (co + 1) * P)
                        for hf in range(2):
                            bs = slice(hf * 512, (hf + 1) * 512)
                            pt = ps_big.tile([P, 512], F32, tag="big")
                            for ci in range(HB):
                                te.matmul(pt[:], wtext[:, ci, cs],
                                          tct[:, ci, bs], start=(ci == 0),
                                          stop=(ci == HB - 1))
                            ve.tensor_scalar(textT[:, co, bs], pt[:],
                                             btext[:, co:co + 1], None, ALU.add)
                            pt2 = ps_big.tile([P, 512], F32, tag="big")
                            for ci in range(HB):
                                te.matmul(pt2[:], wimg[:, ci, cs],
                                          imgsum[:, ci, bs], start=(ci == 0),
                                          stop=(ci == HB - 1))
                            sc.activation(imgT[:, co, bs], pt2[:], AF.Copy,
                                          scale=1.0 / 3.0)
                            pt3 = ps_mid.tile([P, 256], F32, tag="mid")
                            te.matmul(pt3[:], wbf[:, cs],
                                      bizf[:, hf * 512:hf * 512 + 256],
                                      start=True, stop=True)
                            pt4 = ps_mid.tile([P, 256], F32, tag="mid")
                            te.matmul(pt4[:], wbf[:, cs],
                                      bizf[:, hf * 512 + 256:(hf + 1) * 512],
                                      start=True, stop=True)
                            ve.tensor_scalar(sT[:, co, hf * 512:hf * 512 + 256],
                                             pt3[:], bbf[:, co:co + 1],
                                             bimg[:, co:co + 1], ALU.add, ALU.add)
                            ve.tensor_scalar(sT[:, co, hf * 512 + 256:(hf + 1) * 512],
                                             pt4[:], bbf[:, co:co + 1],
                                             bimg[:, co:co + 1], ALU.add, ALU.add)
                        ve.tensor_tensor(sT[:, co, :], sT[:, co, :],
                                         textT[:, co, :], op=ALU.add)
                        ve.tensor_tensor(sT[:, co, :], sT[:, co, :],
                                         imgT[:, co, :], op=ALU.add)
                    # sT = text_emb + (img/3) + bimg + meta + bbf; imgT holds
                    # img/3 -- add bimg to the fusion-local slice only:
                    for co in range(HB):
                        ve.tensor_scalar(imgT[:, co, 0:BSH], imgT[:, co, 0:BSH],
                                         bimg[:, co:co + 1], None, ALU.add)

                    if debug:
                        dsT = ep.tile([P, HB, B], F32, tag="dsT", bufs=1)
                        ve.tensor_copy(dsT[:], sT[:])
                        sy.dma_start(dbg["sT"][:], dsT[:])

                    # s row-major -> s_dram
                    for j in range(BB):
                        srow = ep2.tile([P, H], BF16, tag="srow")
                        for ci in range(HB):
                            ptt = ps_sml.tile([P, P], BF16, tag="sml")
                            te.transpose(ptt[:], sT[:, ci, j * P:(j + 1) * P],
                                         ident[:])
                            if ci % 2 == 0:
                                ve.tensor_copy(srow[:, ci * P:(ci + 1) * P], ptt[:])
                            else:
                                sc.activation(srow[:, ci * P:(ci + 1) * P],
                                              ptt[:], AF.Copy)
                        gp.dma_start(s_dram[j * P:(j + 1) * P, :], srow[:])

                with tc.tile_pool(name="xp2", bufs=1) as xp2:
                    xT = xp2.tile([P, HB, N], BF16, tag="xT")

                    # ====== phase X: build x^T ======
                    with tc.tile_pool(name="xb", bufs=1) as xp:
                        jlidx = xp.tile([P, NB // 16], I16, tag="jlidx")
                        sy.dma_start(jlidx[:], t_jlw[:])
                        umask = xp.tile([P, NU], BF16, tag="umask")
                        sy.dma_start(umask[:], t_um[:])
                        bmask = xp.tile([P, NB], BF16, tag="bmask")
                        sy.dma_start(bmask[:], t_bm[:])

                        ut = xp.tile([P, HB, NU], BF16, tag="ut")
                        gp.dma_start(ut[:],
                                     t_usertT[:].rearrange("(a p) n -> p a n", p=P))
                        for c in range(HB):
                            ve.tensor_tensor(xT[:, c, 0:NU], ut[:, c, :], umask[:],
                                             op=ALU.mult)
                        sg = xp.tile([P, HB, NB], BF16, tag="sgath")
                        gp.dma_gather(sg[:], s_dram[:], jlidx[:], num_idxs=NB,
                                      num_idxs_reg=NB, elem_size=H, transpose=True,
                                      single_packet=False)
                        bt = xp.tile([P, HB, NB], BF16, tag="bt")
                        gp.dma_start(bt[:],
                                     t_biztT[:].rearrange("(a p) n -> p a n", p=P))
                        for c in range(HB):
                            ve.tensor_tensor(sg[:, c, :], sg[:, c, :], bt[:, c, :],
                                             op=ALU.add)
                            ve.tensor_tensor(xT[:, c, NU:N], sg[:, c, :], bmask[:],
                                             op=ALU.mult)
                        if debug:
                            for c in range(HB):
                                dxT = xp.tile([P, N], F32, tag="dxT", bufs=1)
                                ve.tensor_copy(dxT[:], xT[:, c, :])
                                sy.dma_start(dbg["xT"][:, c, :], dxT[:])

                    # ====== layer 1 ======
                    with (tc.tile_pool(name="l1", bufs=1) as l1p,
                          tc.tile_pool(name="l1d", bufs=2) as l1d,
                          tc.tile_pool(name="l1t", bufs=3) as l1t):
                        w1 = l1p.tile([P, HB, H], BF16, tag="w1")
                        gp.dma_start(w1[:], t_w1[:].rearrange("(a p) c -> p a c", p=P))
                        a1 = l1p.tile([P, HB, 2], BF16, tag="a1")
                        gp.dma_start(a1[:], t_a1[:].rearrange("(a p) c -> p a c", p=P))
                        w1T = l1p.tile([P, HB, H], BF16, tag="w1T")
                        gp.dma_start(w1T[:], t_w1T[:].rearrange("(a p) c -> p a c", p=P))
                        s1idx = l1p.tile([P, T1 * 8], I16, tag="s1idx")
                        sy.dma_start(s1idx[:], t_s1w[:])
                        d1idx = l1p.tile([P, T1 * 8], I16, tag="d1idx")
                        sy.dma_start(d1idx[:], t_d1w[:])
                        d1col = l1p.tile([P, T1], F32, tag="d1col")
                        sy.dma_start(d1col[:], t_d1col[:])

                        ws1 = l1p.tile([P, HB, 2], BF16, tag="ws1")
                        for f in range(HB):
                            pw = ps_sml.tile([P, 2], F32, tag="sml")
                            for co in range(HB):
                                te.matmul(pw[:], w1T[:, co, f * P:(f + 1) * P],
                                          a1[:, co, :], start=(co == 0),
                                          stop=(co == HB - 1))
                            ve.tensor_copy(ws1[:, f, :], pw[:])

                        svec = l1p.tile([P, NBLK, 2], F32, tag="svec")
                        for nb in range(NBLK):
                            pv = ps_sml.tile([P, 2], F32, tag="sml")
                            for ci in range(HB):
                                te.matmul(pv[:], xT[:, ci, nb * P:(nb + 1) * P],
                                          ws1[:, ci, :], start=(ci == 0),
                                          stop=(ci == HB - 1))
                            ve.tensor_copy(svec[:, nb, :], pv[:])
                        if debug:
                            sy.dma_start(dbg["svec"][:], svec[:])

                        fat_sb = l1p.tile([P, NBLK, 64], F32, tag="fat_sb")
                        ve.memset(fat_sb[:], 0.0)
                        ve.tensor_copy(fat_sb[:, :, 0:2], svec[:])
                        gp.dma_start(fat_dram[:].rearrange("(a p) c -> p a c", p=P),
                                     fat_sb[:])

                        # h = x @ W1_k, streamed to DRAM
                        for nb in range(NBLK):
                            ph1 = ps_big.tile([P, 512], F32, tag="big")
                            ph2 = ps_mid.tile([P, 256], F32, tag="mid")
                            for ci in range(HB):
                                te.matmul(ph1[:], xT[:, ci, nb * P:(nb + 1) * P],
                                          w1[:, ci, 0:512], start=(ci == 0),
                                          stop=(ci == HB - 1))
                            for ci in range(HB):
                                te.matmul(ph2[:], xT[:, ci, nb * P:(nb + 1) * P],
                                          w1[:, ci, 512:H], start=(ci == 0),
                                          stop=(ci == HB - 1))
                            hst = l1t.tile([P, H], BF16, tag="hst")
                            sc.activation(hst[:, 0:512], ph1[:], AF.Copy)
                            ve.tensor_copy(hst[:, 512:H], ph2[:])
                            sy.dma_start(h_dram[nb * P:(nb + 1) * P, :], hst[:])

                        # --- edge score phase ---
                        ee = l1p.tile([P, T1], F32, tag="ee")
                        ngrp = 8
                        gbounds = [(i * (NBLK // ngrp), (i + 1) * (NBLK // ngrp))
                                   for i in range(ngrp)]
                        for g0, g1 in gbounds:
                            o0, o1 = int(off1[g0]), int(off1[g1])
                            cnt = o1 - o0
                            gs = l1d.tile([P, cnt, 64], F32, tag="fatg")
                            gp.dma_gather(gs[:], fat_dram[:],
                                          s1idx[:, o0 * 8:o1 * 8],
                                          num_idxs=cnt * P, num_idxs_reg=cnt * P,
                                          elem_size=64, single_packet=False)
                            gd = l1d.tile([P, cnt, 64], F32, tag="fatg2")
                            gp.dma_gather(gd[:], fat_dram[:],
                                          d1idx[:, o0 * 8:o1 * 8],
                                          num_idxs=cnt * P, num_idxs_reg=cnt * P,
                                          elem_size=64, single_packet=False)
                            ve.tensor_tensor(ee[:, o0:o1], gs[:, :, 0], gd[:, :, 1],
                                             op=ALU.add)
                            et = l1t.tile([P, cnt], F32, tag="et")
                            ve.tensor_scalar(et[:], ee[:, o0:o1], 0.2, None, ALU.mult)
                            ve.tensor_tensor(ee[:, o0:o1], ee[:, o0:o1], et[:],
                                             op=ALU.max)
                            sc.activation(ee[:, o0:o1], ee[:, o0:o1], AF.Exp)
                        if debug:
                            sy.dma_start(dbg["ee"][:], ee[:])

                        den = l1p.tile([P, NBLK], F32, tag="den")
                        recip = l1p.tile([P, NBLK], F32, tag="recip")
                        if has_b1:
                            b1b = l1p.tile([P, H], F32, tag="b1b")
                            sy.dma_start(b1b[:], t_b1b[:])

                        x2T = xp2.tile([P, HB, N], BF16, tag="xT")  # reuse slot
                        for dpair in range(NBLK // 2):
                            dlo = 2 * dpair
                            o_p = int(off1[dlo])
                            cnt_p = int(off1[dlo + 2] - o_p)
                            gh = l1d.tile([P, 2 * mx1, H], BF16, tag="gh")
                            gp.dma_gather(gh[:, 0:cnt_p, :], h_dram[:],
                                          s1idx[:, o_p * 8:(o_p + cnt_p) * 8],
                                          num_idxs=cnt_p * P, num_idxs_reg=cnt_p * P,
                                          elem_size=H, single_packet=False)
                            for d in (dlo, dlo + 1):
                                nblk = nblk1[d]
                                o = int(off1[d])
                                og = o - o_p
                                mbe = l1d.tile([P, mx1, P], BF16, tag="mbe")
                                for b in range(nblk):
                                    ve.tensor_scalar(mbe[:, b, :], irow[:],
                                                     d1col[:, o + b:o + b + 1],
                                                     ee[:, o + b:o + b + 1],
                                                     ALU.is_equal, ALU.mult)
                                pden = ps_sml.tile([P, 2], F32, tag="sml")
                                for b in range(nblk):
                                    te.matmul(pden[:, 0:1], mbe[:, b, :], ones[:],
                                              start=(b == 0), stop=(b == nblk - 1))
                                ve.tensor_scalar(den[:, d:d + 1], pden[:, 0:1],
                                                 1e-16, None, ALU.add)
                                ve.reciprocal(recip[:, d:d + 1], den[:, d:d + 1])
                                pb1 = ps_big.tile([P, 512], F32, tag="big")
                                pb2 = ps_mid.tile([P, 256], F32, tag="mid")
                                for b in range(nblk):
                                    te.matmul(pb1[:], mbe[:, b, :],
                                              gh[:, og + b, 0:512],
                                              start=(b == 0), stop=(b == nblk - 1))
                                for b in range(nblk):
                                    te.matmul(pb2[:], mbe[:, b, :],
                                              gh[:, og + b, 512:H],
                                              start=(b == 0), stop=(b == nblk - 1))
                                x2st = l1t.tile([P, H], BF16, tag="hst")
                                if has_b1:
                                    tmp = l1t.tile([P, H], F32, tag="tmpb")
                                    ve.tensor_scalar(tmp[:, 0:512], pb1[:],
                                                     recip[:, d:d + 1], None,
                                                     ALU.mult)
                                    ve.tensor_scalar(tmp[:, 512:H], pb2[:],
                                                     recip[:, d:d + 1], None,
                                                     ALU.mult)
                                    ve.tensor_tensor(tmp[:], tmp[:], b1b[:],
                                                     op=ALU.add)
                                    ve.tensor_scalar(x2st[:], tmp[:], 0.0, None,
                                                     ALU.max)
                                else:
                                    sc.activation(x2st[:, 0:512], pb1[:], AF.Relu,
                                                  scale=recip[:, d:d + 1])
                                    ve.tensor_scalar(x2st[:, 512:H], pb2[:],
                                                     recip[:, d:d + 1], 0.0,
                                                     ALU.mult, ALU.max)
                                for c in range(HB):
                                    ptt = ps_sml.tile([P, P], BF16, tag="sml")
                                    te.transpose(ptt[:], x2st[:, c * P:(c + 1) * P],
                                                 ident[:])
                                    if c % 2 == 0:
                                        ve.tensor_copy(
                                            x2T[:, c, d * P:(d + 1) * P], ptt[:])
                                    else:
                                        sc.activation(
                                            x2T[:, c, d * P:(d + 1) * P],
                                            ptt[:], AF.Copy)
                        if debug:
                            sy.dma_start(dbg["den"][:], den[:])
                            for c in range(HB):
                                dx2T = l1d.tile([P, N], F32, tag="dx2T", bufs=1)
                                ve.tensor_copy(dx2T[:], x2T[:, c, :])
                                sy.dma_start(dbg["x2T"][:, c, :], dx2T[:])

                    # ====== layer 2 local matmul ======
                    with tc.tile_pool(name="l2m", bufs=1) as l2m:
                        w2 = l2m.tile([P, HB, H], BF16, tag="w2")
                        gp.dma_start(w2[:], t_w2[:].rearrange("(a p) c -> p a c", p=P))
                        a2 = l2m.tile([P, HB, 2], BF16, tag="a2")
                        gp.dma_start(a2[:], t_a2[:].rearrange("(a p) c -> p a c", p=P))
                        w2T = l2m.tile([P, HB, H], BF16, tag="w2T")
                        gp.dma_start(w2T[:], t_w2T[:].rearrange("(a p) c -> p a c", p=P))

                        ws2 = l2m.tile([P, HB, 2], BF16, tag="ws2")
                        for f in range(HB):
                            pw = ps_sml.tile([P, 2], F32, tag="sml")
                            for co in range(HB):
                                te.matmul(pw[:], w2T[:, co, f * P:(f + 1) * P],
                                          a2[:, co, :], start=(co == 0),
                                          stop=(co == HB - 1))
                            ve.tensor_copy(ws2[:, f, :], pw[:])

                        for nb in range(NBLK):
                            ph1 = ps_big.tile([P, 512], F32, tag="big")
                            ph2 = ps_mid.tile([P, 256], F32, tag="mid")
                            pv = ps_sml.tile([P, 2], F32, tag="sml")
                            for ci in range(HB):
                                te.matmul(ph1[:], x2T[:, ci, nb * P:(nb + 1) * P],
                                          w2[:, ci, 0:512], start=(ci == 0),
                                          stop=(ci == HB - 1))
                            for ci in range(HB):
                                te.matmul(ph2[:], x2T[:, ci, nb * P:(nb + 1) * P],
                                          w2[:, ci, 512:H], start=(ci == 0),
                                          stop=(ci == HB - 1))
                            for ci in range(HB):
                                te.matmul(pv[:], x2T[:, ci, nb * P:(nb + 1) * P],
                                          ws2[:, ci, :], start=(ci == 0),
                                          stop=(ci == HB - 1))
                            ast = l2m.tile([P, ARW], BF16, tag="ast", bufs=3)
                            ve.memset(ast[:, 770:ARW], 0.0)
                            sc.activation(ast[:, 0:512], ph1[:], AF.Copy)
                            ve.tensor_copy(ast[:, 512:H], ph2[:])
                            ve.tensor_copy(ast[:, H:770], pv[:])
                            sy.dma_start(ar_in[nb * P:(nb + 1) * P, :], ast[:])

                # ====== collectives + L2 edge + fusion ======
                with (tc.tile_pool(name="fu", bufs=1) as fup,
                      tc.tile_pool(name="fud", bufs=2) as fud,
                      tc.tile_pool(name="fut", bufs=3) as fut):
                    wf1a = fup.tile([P, F2 // P, F2], BF16, tag="wf1a")
                    sy.dma_start(wf1a[:],
                                 t_wf1[0:F2, :].rearrange("(a p) c -> p a c", p=P))
                    wf1b = fup.tile([P, F2 // P, F2], BF16, tag="wf1b")
                    sy.dma_start(wf1b[:],
                                 t_wf1[F2:F4, :].rearrange("(a p) c -> p a c", p=P))
                    wf2 = fup.tile([P, F2 // P, H], BF16, tag="wf2")
                    sy.dma_start(wf2[:],
                                 t_wf2[:].rearrange("(a p) c -> p a c", p=P))
                    wf3 = fup.tile([P, HB, 1], BF16, tag="wf3")
                    sy.dma_start(wf3[:], t_wf3[:].rearrange("(a p) c -> p a c", p=P))
                    bf1 = fup.tile([P, F2 // P], F32, tag="bf1")
                    sy.dma_start(bf1[:], t_bf1[:].rearrange("(a p) -> p a", p=P))
                    bf2 = fup.tile([P, HB], F32, tag="bf2")
                    sy.dma_start(bf2[:], t_bf2[:].rearrange("(a p) -> p a", p=P))
                    s2idx = fup.tile([P, T2 * 8], I16, tag="s2idx")
                    sy.dma_start(s2idx[:], t_s2w[:])
                    d2idx = fup.tile([P, T2 * 8], I16, tag="d2idx")
                    sy.dma_start(d2idx[:], t_d2w[:])
                    d2col = fup.tile([P, T2], F32, tag="d2col")
                    sy.dma_start(d2col[:], t_d2col[:])
                    pu = fup.tile([P, 2, P], BF16, tag="pu")
                    sy.dma_start(pu[:], t_pu[:])
                    pbm = fup.tile([P, 2, P], BF16, tag="pbm")
                    sy.dma_start(pbm[:], t_pb[:])

                    gp.collective_compute("ReduceScatter", ALU.add,
                                          replica_groups=rg,
                                          ins=[ar_in.opt()], outs=[arrs.opt()])
                    gp.collective_compute("AllGather", ALU.bypass,
                                          replica_groups=rg,
                                          ins=[arrs.opt()], outs=[ar_out.opt()])
                    if debug:
                        for c in range(HB):
                            dar = fud.tile([P, NBLK, P], BF16, tag="dar", bufs=1)
                            gp.dma_start(
                                dar[:],
                                ar_out[:, c * P:(c + 1) * P].rearrange(
                                    "(a p) c -> p a c", p=P))
                            gp.dma_start(
                                dbg["ar"][:, c * P:(c + 1) * P].rearrange(
                                    "(a p) c -> p a c", p=P), dar[:])
                        dar2 = fud.tile([P, NBLK, ARW - H], BF16, tag="dar2",
                                        bufs=1)
                        gp.dma_start(dar2[:],
                                     ar_out[:, H:ARW].rearrange(
                                         "(a p) c -> p a c", p=P))
                        gp.dma_start(dbg["ar"][:, H:ARW].rearrange(
                            "(a p) c -> p a c", p=P), dar2[:])

                    # --- layer 2 edge phase (fusion-aligned slots) ---
                    svec2 = fup.tile([P, NBLK, 2], F32, tag="svec2")
                    s2bf = fud.tile([P, NBLK, 2], BF16, tag="s2bf")
                    sy.dma_start(s2bf[:],
                                 ar_out[:, H:770].rearrange("(a p) c -> p a c", p=P))
                    ve.tensor_copy(svec2[:], s2bf[:])
                    fat2_sb = fup.tile([P, NBLK, 64], F32, tag="fat_sb")
                    ve.memset(fat2_sb[:], 0.0)
                    ve.tensor_copy(fat2_sb[:, :, 0:2], svec2[:])
                    gp.dma_start(fat2_dram[:].rearrange("(a p) c -> p a c", p=P),
                                 fat2_sb[:])

                    ee2 = fup.tile([P, T2], F32, tag="ee2")
                    gs2 = fud.tile([P, T2, 64], F32, tag="fatg")
                    gp.dma_gather(gs2[:], fat2_dram[:], s2idx[:], num_idxs=T2 * P,
                                  num_idxs_reg=T2 * P, elem_size=64,
                                  single_packet=False)
                    gd2 = fud.tile([P, T2, 64], F32, tag="fatg2")
                    gp.dma_gather(gd2[:], fat2_dram[:], d2idx[:], num_idxs=T2 * P,
                                  num_idxs_reg=T2 * P, elem_size=64,
                                  single_packet=False)
                    ve.tensor_tensor(ee2[:], gs2[:, :, 0], gd2[:, :, 1], op=ALU.add)
                    et2 = fup.tile([P, T2], F32, tag="et2")
                    ve.tensor_scalar(et2[:], ee2[:], 0.2, None, ALU.mult)
                    ve.tensor_tensor(ee2[:], ee2[:], et2[:], op=ALU.max)
                    sc.activation(ee2[:], ee2[:], AF.Exp)

                    den2 = fup.tile([P, 2], F32, tag="den2")
                    recip2 = fup.tile([P, 2], F32, tag="recip2")
                    if has_b2:
                        b2b = fup.tile([P, H], F32, tag="b2b")
                        sy.dma_start(b2b[:], t_b2b[:])

                    xo0 = fup.tile([P, H], BF16, tag="xo0")
                    xo1 = fup.tile([P, H], BF16, tag="xo1")
                    for sb, xo in ((0, xo0), (1, xo1)):
                        o = sb * nblk2u
                        gh = fud.tile([P, nblk2u, H], BF16, tag="gh2")
                        gp.dma_gather(gh[:], ar_out[:, 0:H],
                                      s2idx[:, o * 8:(o + nblk2u) * 8],
                                      num_idxs=nblk2u * P,
                                      num_idxs_reg=nblk2u * P,
                                      elem_size=H, elem_step=ARW,
                                      single_packet=False)
                        mbe = fud.tile([P, nblk2u, P], BF16, tag="mbe2")
                        for b in range(nblk2u):
                            ve.tensor_scalar(mbe[:, b, :], irow[:],
                                             d2col[:, o + b:o + b + 1],
                                             ee2[:, o + b:o + b + 1],
                                             ALU.is_equal, ALU.mult)
                        pden = ps_sml.tile([P, 2], F32, tag="sml")
                        for b in range(nblk2u):
                            te.matmul(pden[:, 0:1], mbe[:, b, :], ones[:],
                                      start=(b == 0), stop=(b == nblk2u - 1))
                        ve.tensor_scalar(den2[:, sb:sb + 1], pden[:, 0:1],
                                         1e-16, None, ALU.add)
                        ve.reciprocal(recip2[:, sb:sb + 1], den2[:, sb:sb + 1])
                        pb1 = ps_big.tile([P, 512], F32, tag="big")
                        pb2 = ps_mid.tile([P, 256], F32, tag="mid")
                        for b in range(nblk2u):
                            te.matmul(pb1[:], mbe[:, b, :], gh[:, b, 0:512],
                                      start=(b == 0), stop=(b == nblk2u - 1))
                        for b in range(nblk2u):
                            te.matmul(pb2[:], mbe[:, b, :], gh[:, b, 512:H],
                                      start=(b == 0), stop=(b == nblk2u - 1))
                        if has_b2:
                            tmp = fut.tile([P, H], F32, tag="tmpb")
                            ve.tensor_scalar(tmp[:, 0:512], pb1[:],
                                             recip2[:, sb:sb + 1], None, ALU.mult)
                            ve.tensor_scalar(tmp[:, 512:H], pb2[:],
                                             recip2[:, sb:sb + 1], None, ALU.mult)
                            ve.tensor_tensor(xo[:], tmp[:], b2b[:], op=ALU.add)
                        else:
                            sc.activation(xo[:, 0:512], pb1[:], AF.Copy,
                                          scale=recip2[:, sb:sb + 1])
                            ve.tensor_scalar(xo[:, 512:H], pb2[:],
                                             recip2[:, sb:sb + 1], None, ALU.mult)
                        if debug:
                            dxo = fut.tile([P, H], F32, tag="dxo")
                            ve.tensor_copy(dxo[:], xo[:])
                            sy.dma_start(dbg["xo"][:, sb, :], dxo[:])

                    # permutation matmuls -> xuT / xbT
                    xuT = fup.tile([P, HB, BSH], BF16, tag="xuT")
                    xbT = fup.tile([P, HB, BSH], BF16, tag="xbT")
                    for c in range(HB):
                        cs = slice(c * P, (c + 1) * P)
                        pf = ps_sml.tile([P, P], F32, tag="sml")
                        te.matmul(pf[:], xo0[:, cs], pu[:, 0, :], start=True,
                                  stop=False)
                        te.matmul(pf[:], xo1[:, cs], pu[:, 1, :], start=False,
                                  stop=True)
                        ve.tensor_copy(xuT[:, c, :], pf[:])
                        pg = ps_sml.tile([P, P], F32, tag="sml")
                        te.matmul(pg[:], xo0[:, cs], pbm[:, 0, :], start=True,
                                  stop=False)
                        te.matmul(pg[:], xo1[:, cs], pbm[:, 1, :], start=False,
                                  stop=True)
                        ve.tensor_copy(xbT[:, c, :], pg[:])
                    if debug:
                        duT = fud.tile([P, HB, BSH], F32, tag="duT", bufs=1)
                        ve.tensor_copy(duT[:], xuT[:])
                        sy.dma_start(dbg["xuT"][:], duT[:])
                        dbT = fud.tile([P, HB, BSH], F32, tag="dbT", bufs=1)
                        ve.tensor_copy(dbT[:], xbT[:])
                        sy.dma_start(dbg["xbT"][:], dbT[:])

                    # ====== fusion MLP (local batch = cols 0:128) ======
                    cat_a = [xuT, xbT]
                    h1fT = fup.tile([P, F2 // P, BSH], BF16, tag="h1fT")
                    for ob in range(F2 // P):
                        obs = slice(ob * P, (ob + 1) * P)
                        pf = ps_sml.tile([P, BSH], F32, tag="sml")
                        for fb in range(F2 // P):
                            rhs = cat_a[fb // HB][:, fb % HB, :]
                            te.matmul(pf[:], wf1a[:, fb, obs], rhs,
                                      start=(fb == 0), stop=False)
                        for fb in range(F2 // P):
                            if fb < HB:
                                rhs = textT[:, fb, 0:BSH]
                            else:
                                rhs = imgT[:, fb - HB, 0:BSH]
                            te.matmul(pf[:], wf1b[:, fb, obs], rhs,
                                      start=False, stop=(fb == F2 // P - 1))
                        ve.tensor_scalar(h1fT[:, ob, :], pf[:],
                                         bf1[:, ob:ob + 1], 0.0, ALU.add,
                                         ALU.max)

                    h2fT = fup.tile([P, HB, BSH], BF16, tag="h2fT")
                    for ob in range(HB):
                        obs = slice(ob * P, (ob + 1) * P)
                        pf = ps_sml.tile([P, BSH], F32, tag="sml")
                        for fb in range(F2 // P):
                            te.matmul(pf[:], wf2[:, fb, obs], h1fT[:, fb, :],
                                      start=(fb == 0), stop=(fb == F2 // P - 1))
                        ve.tensor_scalar(h2fT[:, ob, :], pf[:],
                                         bf2[:, ob:ob + 1], 0.0, ALU.add,
                                         ALU.max)

                    py = ps_sml.tile([P, 2], F32, tag="sml")
                    for c in range(HB):
                        te.matmul(py[:, 0:1], h2fT[:, c, :], wf3[:, c, :],
                                  start=(c == 0), stop=(c == HB - 1))
                    ysb = fup.tile([P, 1], F32, tag="ysb")
                    ve.tensor_scalar(ysb[:], py[:, 0:1], pr["bf3_val"], None,
                                     ALU.add)
                    sy.dma_start(t_y[:], ysb[:])

    nc.compile()
    return nc


def make_in_maps(pr):
    inp = pr["inp"]
    f32 = np.float32

    def bf(x):
        return np.ascontiguousarray(np.asarray(x), dtype=np.float32).astype(_nbf)

    usertT = bf(inp["user_table"].T)
    biztT = bf(inp["biz_table"].T)
    a2 = bf(np.stack([inp["att_src2"][0], inp["att_dst2"][0]], axis=1))
    wf1 = bf(inp["Wf1"])
    wf2 = bf(inp["Wf2"])
    wf3 = bf(inp["Wf3"])
    wtext = bf(inp["W_text"])
    wimg = bf(inp["W_img"])
    wbf = bf(inp["W_bf"])
    in_maps = []
    for k in range(NCORES):
        rot = np.roll(np.arange(B), -k * BSH)
        m = dict(
            text_clsT=bf(inp["text_cls"][rot].T),
            imgT=bf(inp["img_cls"][rot].transpose(1, 2, 0)),
            bizfT=bf(inp["biz_feats"][rot].T),
            W_text=wtext, W_img=wimg, W_bf=wbf,
            b_text=inp["b_text"].astype(f32),
            b_img=inp["b_img"].astype(f32),
            b_bf=inp["b_bf"].astype(f32),
            user_tableT=usertT,
            biz_tableT=biztT,
            W1k=bf(inp["W1"][:, k * H:(k + 1) * H]),
            W1kT=bf(inp["W1"][:, k * H:(k + 1) * H].T),
            W2k=bf(inp["W2"][k * H:(k + 1) * H, :]),
            W2kT=bf(inp["W2"][k * H:(k + 1) * H, :].T),
            a1k=bf(np.stack([inp["att_src1"][k], inp["att_dst1"][k]], axis=1)),
            a2=a2,
            Wf1=wf1, Wf2=wf2, Wf3=wf3,
            bf1=inp["bf1"].astype(f32),
            bf2=inp["bf2"].astype(f32),
            src1w=pr["src1w"], dst1w=pr["dst1w"], d1col=pr["d1col"],
            src2w=pr["l2"][k]["src2w"], dst2w=pr["l2"][k]["dst2w"],
            d2col=pr["l2"][k]["d2col"],
            PU=pr["l2"][k]["PU"], PB=pr["l2"][k]["PB"],
            jlw=pr["l2"][k]["jlw"],
            u_mask_b=pr["u_mask_b"], bm025_b=pr["bm025_b"],
            ident=pr["ident"], irow=pr["irow"],
        )
        if pr["has_b1"]:
            m["b1_b"] = np.broadcast_to(
                inp["b1"][k * H:(k + 1) * H].astype(f32), (P, H)).copy()
        if pr["has_b2"]:
            m["b2_b"] = np.broadcast_to(inp["b2"].astype(f32), (P, H)).copy()
        in_maps.append(m)
    return in_maps


def run(inputs, debug=False, want_results=False, reps=1):
    pr = host_prep(inputs)
    nc = build_program(pr, debug=debug, reps=reps)
    in_maps = make_in_maps(pr)
    res = bass_utils.run_bass_kernel_spmd(
        nc, in_maps, core_ids=list(range(NCORES)), trace=False)
    y = np.concatenate([res.results[k]["y"][:, 0] for k in range(NCORES)])
    if want_results:
        return y.astype(np.float32), res, pr, nc, in_maps
    return y.astype(np.float32)


def kernel(**inputs):
    return run(inputs)
